# revision 1
# baseline (speedup 1.0000x reference)
"""Multi-head causal attention (B=2, S=2048, D=1024, H=16, Dh=64) on 8 TRN2
NeuronCores.

Sharding: tensor-parallel over heads — core c owns heads (2c, 2c+1).

v2: single merged pipeline. Per 512-token chunk u: QKV-project the chunk
(contraction-tiled matmuls), transpose V, then immediately run causal
attention chunks for head 0 (eager) and head 1 (lagged) of that query block.
Interleaving projection matmuls between attention chunks keeps the PE stream
gapless, which matters because the cost model ramps the PE clock
(0.65 -> 1.2 -> 2.4 GHz after 3us of *continuous* execution) and any idle
gap resets the ramp.  Softmax normalization runs off the critical engines:
DVE reciprocal of the denominator row (free 65th row of the PV matmul via a
ones-column in V), gpsimd partition-broadcast, DVE multiply straight out of
PSUM.  Exp stays on ACT (the only transcendental engine).  The two
all-to-alls (one per head) are staggered: head-0's collective issues while
head-1's lagged attention chunks still run; the output projection is split
into a head-0 half (runs during head-1's collective) and a head-1 half.
All HBM loads are few big contiguous DMAs (1 per x-chunk / weight tensor).
"""
import ml_dtypes
import numpy as np

import concourse.bass as bass
import concourse.mybir as mybir
import concourse.tile as tile
from concourse.bass_utils import run_bass_kernel_spmd

F32 = mybir.dt.float32
BF16 = mybir.dt.bfloat16

B = 2
S = 2048
D = 1024
H = 16
DH = 64
N_CORES = 8
R = B * S          # 4096 global rows
RC = R // N_CORES  # 512 rows per core for the output projection
NT = R // 512      # 8 token chunks
NC_T = D // 128    # 8 contraction tiles

# ---------------------------------------------------------------------------
# BIR splitter: this toolchain's walrus rejects >1 sem-wait per instruction;
# move extra waits onto preceding same-engine nops (identical semantics).
def _split_waits(nc, maxw=1):
    for f in nc.m.functions:
        for bb in f.blocks:
            new_insts = []
            for ins in bb.instructions:
                si = ins.sync_info
                waits = list(si.on_wait) if si and si.on_wait else []
                if len(waits) > maxw:
                    carry, keep = waits[:-maxw], waits[-maxw:]
                    for j in range(0, len(carry), maxw):
                        new_insts.append(
                            mybir.InstNoOp(
                                name=f"{ins.name}-ws{j}",
                                engine=ins.engine,
                                sync_info=mybir.SyncInfo(
                                    on_wait=carry[j : j + maxw], on_update=[]
                                ),
                                bass_nofuse=True,
                            )
                        )
                    ins.sync_info = mybir.SyncInfo(
                        on_wait=keep,
                        on_update=list(si.on_update) if si.on_update else [],
                    )
                new_insts.append(ins)
            bb.instructions = new_insts


def _build():
    nc = bass.Bass()

    xT_d = nc.declare_dram_parameter("xT", [128, R * NC_T], BF16, isOutput=False)
    wT_d = nc.declare_dram_parameter("wT", [128, NC_T * 6 * DH], BF16, isOutput=False)
    woT_d = nc.declare_dram_parameter("woT", [128, NC_T * D], BF16, isOutput=False)
    masks_d = nc.declare_dram_parameter("masks", [128, 2048], BF16, isOutput=False)
    ident_d = nc.declare_dram_parameter("ident", [128, 128], BF16, isOutput=False)
    ones_d = nc.declare_dram_parameter("ones", [1, 64], mybir.dt.float32, isOutput=False)
    out_d = nc.declare_dram_parameter("out", [RC, D], F32, isOutput=True)

    a2a_in = [
        nc.dram_tensor(f"a2a_in{h}", [N_CORES, 64, RC], BF16) for h in range(2)
    ]
    a2a_out = [
        nc.dram_tensor(f"a2a_out{h}", [N_CORES, 64, RC], BF16) for h in range(2)
    ]

    with tile.TileContext(nc) as tc:
      with nc.allow_low_precision(reason="bf16 attention pipeline"):
        with (
            tc.tile_pool(name="main", bufs=1) as main,
            tc.tile_pool(name="xs", bufs=3) as x_pool,
            tc.tile_pool(name="vt", bufs=2) as vt_pool,
            tc.tile_pool(name="work", bufs=6) as work,
            tc.tile_pool(name="norm", bufs=2) as norm_pool,
            tc.tile_pool(name="outp", bufs=2) as out_pool,
        ):
            # ---- big contiguous loads ---------------------------------------
            wbig = main.tile([128, NC_T * 6 * DH], BF16, tag="wbig")
            nc.sync.dma_start(
                out=wbig, in_=wT_d[:, :]
            )

            xbig = [None] * NT

            def issue_x(u):
                xt = x_pool.tile([128, 4096], BF16, tag="x", name=f"x{u}")
                nc.sync.dma_start(
                    out=xt, in_=xT_d[:, 4096 * u : 4096 * (u + 1)]
                )
                xbig[u] = xt

            issue_x(0)
            issue_x(1)

            masks_t = main.tile([128, 2048], BF16, tag="masks")
            nc.sync.dma_start(
                out=masks_t, in_=masks_d[:, :]
            )
            ident = main.tile([128, 128], BF16, tag="ident")
            nc.sync.dma_start(out=ident, in_=ident_d[:, :])
            ones_row = main.tile([1, 64], mybir.dt.float32r, tag="ones")
            nc.gpsimd.dma_start(out=ones_row, in_=ones_d[:, :])
            wobig = main.tile([128, NC_T * D], BF16, tag="wobig")
            nc.sync.dma_start(
                out=wobig, in_=woT_d[:, :]
            )

            qT = main.tile([128, R], BF16, tag="qT")
            kT = main.tile([128, R], BF16, tag="kT")
            attnT = [
                main.tile([64, R], BF16, tag=f"attnT{h}", name=f"attnT{h}")
                for h in range(2)
            ]
            v_augs = [
                main.tile([128, 130], BF16, tag=f"va{st}", name=f"va{st}")
                for st in range(4 * NT)
            ]
            for st in range(4 * NT):
                nc.vector.memset(v_augs[st][:, 64:65], 1.0)
                nc.vector.memset(v_augs[st][:, 129:130], 1.0)

            afbig = [None, None]

            with (
                tc.tile_pool(name="psum_qkv", bufs=2, space="PSUM") as psum_qkv,
                tc.tile_pool(name="psum_s", bufs=2, space="PSUM") as psum_s,
                tc.tile_pool(name="psum_pv", bufs=2, space="PSUM") as psum_pv,
            ):

                pending_norm = []

                def flush_norms():
                    while pending_norm:
                        h, u, q0, pv, rec = pending_norm.pop(0)
                        rB = psum_s.tile(
                            [64, 512], F32, tag="sp", name=f"rB{h}{u}"
                        )
                        nc.tensor.matmul(
                            rB, lhsT=ones_row, rhs=rec, start=True, stop=True
                        )
                        rb = norm_pool.tile([64, 512], F32, tag="rb", name=f"rb{h}{u}")
                        nc.vector.tensor_copy(rb, rB)
                        nc.vector.tensor_mul(
                            attnT[h][:, q0 : q0 + 512], pv[0:64, :], rb[0:64, :]
                        )
                        nc.sync.dma_start(
                            out=a2a_in[h][u], in_=attnT[h][:, q0 : q0 + 512]
                        )

                def transpose_pair(u, vtmp, j0):
                    for j in (j0, j0 + 1):
                        pt = psum_s.tile(
                            [128, 128], BF16, tag="sp", name=f"pt{u}_{j}"
                        )
                        nc.tensor.transpose(pt, vtmp[:, 128 * j : 128 * (j + 1)], ident)
                        va = v_augs[4 * u + j]
                        nc.vector.tensor_copy(va[:, 0:64], pt[:, 0:64])
                        nc.vector.tensor_copy(va[:, 65:129], pt[:, 64:128])

                def P(u):
                    xb = xbig[u]
                    # v (mi=2)
                    psv = psum_qkv.tile([128, 512], F32, tag="ps", name=f"psv{u}")
                    for ct in range(NC_T):
                        nc.tensor.matmul(
                            psv,
                            lhsT=wbig[:, 384 * ct + 256 : 384 * ct + 384],
                            rhs=xb[:, 512 * ct : 512 * (ct + 1)],
                            start=(ct == 0),
                            stop=(ct == NC_T - 1),
                        )
                    vtmp = vt_pool.tile([128, 512], BF16, tag="vt", name=f"vt{u}")
                    nc.vector.tensor_copy(vtmp, psv)
                    # k (mi=1)
                    psk = psum_qkv.tile([128, 512], F32, tag="ps", name=f"psk{u}")
                    for ct in range(NC_T):
                        nc.tensor.matmul(
                            psk,
                            lhsT=wbig[:, 384 * ct + 128 : 384 * ct + 256],
                            rhs=xb[:, 512 * ct : 512 * (ct + 1)],
                            start=(ct == 0),
                            stop=(ct == NC_T - 1),
                        )
                    nc.vector.tensor_copy(kT[:, 512 * u : 512 * (u + 1)], psk)
                    # transposes for st 0,1 of this chunk (v evac done during k)
                    transpose_pair(u, vtmp, 0)
                    # q (mi=0), first half
                    psq = psum_qkv.tile([128, 512], F32, tag="ps", name=f"psq{u}")
                    for ct in range(4):
                        nc.tensor.matmul(
                            psq,
                            lhsT=wbig[:, 384 * ct : 384 * ct + 128],
                            rhs=xb[:, 512 * ct : 512 * (ct + 1)],
                            start=(ct == 0),
                            stop=False,
                        )
                    transpose_pair(u, vtmp, 2)
                    for ct in range(4, NC_T):
                        nc.tensor.matmul(
                            psq,
                            lhsT=wbig[:, 384 * ct : 384 * ct + 128],
                            rhs=xb[:, 512 * ct : 512 * (ct + 1)],
                            start=False,
                            stop=(ct == NC_T - 1),
                        )
                    nc.vector.tensor_copy(qT[:, 512 * u : 512 * (u + 1)], psq)
                    if u + 2 < NT:
                        issue_x(u + 2)

                def A(h, u):
                    b, qc = divmod(u, 4)
                    hb = 64 * h
                    q0 = 512 * u
                    nkt = 4 * qc + 4
                    ng = nkt // 2
                    gs = list(range(ng))  # diagonal (masked) groups last
                    es = {}

                    def emit_s(g, split=False):
                        sp = psum_s.tile(
                            [128, 1024], F32, tag="sp", name=f"sp{h}_{u}_{g}"
                        )
                        e2 = work.tile(
                            [128, 1024], BF16, tag="e2", name=f"e{h}_{u}_{g}"
                        )
                        for half in range(2):
                            kt = 2 * g + half
                            k0 = 2048 * b + 128 * kt
                            nc.tensor.matmul(
                                sp[:, 512 * half : 512 * (half + 1)],
                                lhsT=kT[hb : hb + 64, k0 : k0 + 128],
                                rhs=qT[hb : hb + 64, q0 : q0 + 512],
                                start=True,
                                stop=True,
                            )
                            if split:
                                # halve the first group's exp latency so the
                                # first PV never waits on ACT
                                nc.scalar.activation(
                                    e2[:, 512 * half : 512 * (half + 1)],
                                    sp[:, 512 * half : 512 * (half + 1)],
                                    mybir.ActivationFunctionType.Exp,
                                    scale=0.125,
                                )
                        if not split:
                            nc.scalar.activation(
                                e2, sp, mybir.ActivationFunctionType.Exp, scale=0.125
                            )
                        for half in range(2):
                            m = 2 * g + half - 4 * qc
                            if m >= 0:
                                nc.vector.tensor_mul(
                                    e2[:, 512 * half : 512 * (half + 1)],
                                    e2[:, 512 * half : 512 * (half + 1)],
                                    masks_t[:, 512 * m : 512 * (m + 1)],
                                )
                        es[g] = e2

                    emit_s(gs[0], split=True)
                    if ng > 1:
                        emit_s(gs[1])
                    pv = psum_pv.tile([65, 512], F32, tag="pv", name=f"pv{h}_{u}")
                    for i, g in enumerate(gs):
                        e2 = es.pop(g)
                        for half in range(2):
                            kt = 2 * g + half
                            nc.tensor.matmul(
                                pv,
                                lhsT=v_augs[16 * b + kt][:, 65 * h : 65 * h + 65],
                                rhs=e2[:, 512 * half : 512 * (half + 1)],
                                start=(i == 0 and half == 0),
                                stop=(i == len(gs) - 1 and half == 1),
                            )
                        if i == 0:
                            flush_norms()
                        if i + 2 < len(gs):
                            emit_s(gs[i + 2])
                    # normalize, stage 1: fast reciprocal of the denom row
                    # (single custom-DVE op).  The PE-side broadcast + multiply
                    # + store are DEFERRED into the next unit's stream so the
                    # PE never waits on this chain (any PE bubble resets the
                    # clock ramp).
                    lnd = norm_pool.tile([1, 512], F32, tag="lnd", name=f"ln{h}{u}")
                    nc.scalar.activation(
                        lnd, pv[64:65, :], mybir.ActivationFunctionType.Ln
                    )
                    rec = norm_pool.tile(
                        [1, 512], mybir.dt.float32r, tag="rec", name=f"rc{h}{u}"
                    )
                    nc.scalar.activation(
                        rec, lnd, mybir.ActivationFunctionType.Exp, scale=-1.0
                    )
                    pending_norm.append((h, u, q0, pv, rec))

                def CC(h):
                    flush_norms()
                    nc.gpsimd.collective_compute(
                        "AllToAll",
                        mybir.AluOpType.bypass,
                        ins=[a2a_in[h][:]],
                        outs=[a2a_out[h][:]],
                        replica_groups=[list(range(N_CORES))],
                    )

                def AF(h):
                    # load the gathered head off HBM
                    af = main.tile([128, 2048], BF16, tag=f"af{h}", name=f"af{h}")
                    for t in range(4):
                        nc.sync.dma_start(
                            out=af[:, 512 * t : 512 * (t + 1)],
                            in_=a2a_out[h][2 * t : 2 * t + 2].rearrange(
                                "pa b c -> (pa b) c"
                            ),
                        )
                    afbig[h] = af

                partials = {}

                def PH0():
                    for stile in range(RC // 128):
                        for dc in range(2):
                            po = psum_qkv.tile(
                                [128, 512], F32, tag="ps", name=f"poh0{stile}{dc}"
                            )
                            for t in range(4):
                                nc.tensor.matmul(
                                    po,
                                    lhsT=afbig[0][:, 512 * t + 128 * stile : 512 * t + 128 * stile + 128],
                                    rhs=wobig[:, 1024 * t + 512 * dc : 1024 * t + 512 * (dc + 1)],
                                    start=(t == 0),
                                    stop=(t == 3),
                                )
                            part = main.tile(
                                [128, 512], F32, tag=f"ph{stile}{dc}",
                                name=f"ph{stile}{dc}",
                            )
                            nc.vector.tensor_copy(part, po)
                            partials[(stile, dc)] = part

                # ---- merged pipeline schedule --------------------------------
                # h0 eager, h1 lagged; h0's collective issues while h1's tail
                # chunks still occupy the PE.
                order = [
                    ("P", 0), ("A", 0, 0),
                    ("P", 1), ("A", 0, 1), ("A", 1, 0),
                    ("P", 2), ("A", 0, 2), ("A", 1, 1),
                    ("P", 3), ("A", 0, 3), ("A", 1, 2),
                    ("P", 4), ("A", 0, 4),
                    ("P", 5), ("A", 0, 5),
                    ("P", 6), ("A", 0, 6),
                    ("P", 7), ("A", 0, 7), ("CC", 0),
                    ("A", 1, 3), ("A", 1, 4), ("A", 1, 5), ("A", 1, 6),
                    ("A", 1, 7), ("FL",), ("AF", 0), ("PH0",), ("CC", 1),
                ]
                for unit in order:
                    if unit[0] == "P":
                        P(unit[1])
                    elif unit[0] == "A":
                        A(unit[1], unit[2])
                    elif unit[0] == "CC":
                        CC(unit[1])
                    elif unit[0] == "FL":
                        flush_norms()
                    elif unit[0] == "PH0":
                        PH0()
                    else:
                        AF(unit[1])

            # ---- output projection, h1 half: accumulate after cc2, fuse the
            # h0 partials with a DVE add on evacuation
            with tc.tile_pool(name="psum_o", bufs=2, space="PSUM") as psum_o:
                AF(1)
                for stile in range(RC // 128):
                    ot = out_pool.tile([128, D], F32, tag="ot", name=f"ot{stile}")
                    for dc in range(2):
                        po = psum_o.tile(
                            [128, 512], F32, tag="po", name=f"poh1{stile}{dc}"
                        )
                        for t in range(4):
                            nc.tensor.matmul(
                                po,
                                lhsT=afbig[1][:, 512 * t + 128 * stile : 512 * t + 128 * stile + 128],
                                rhs=wobig[:, 1024 * (4 + t) + 512 * dc : 1024 * (4 + t) + 512 * (dc + 1)],
                                start=(t == 0),
                                stop=(t == 3),
                            )
                        nc.vector.tensor_add(
                            ot[:, 512 * dc : 512 * (dc + 1)],
                            po,
                            partials[(stile, dc)],
                        )
                    nc.sync.dma_start(
                        out=out_d[128 * stile : 128 * (stile + 1), :], in_=ot
                    )

    _split_waits(nc, maxw=1)
    return nc


def _install_ntff_shim():
    """Register the NTFF profile hook that this image's `antenv` lacks.

    bass_utils reads `antenv.axon_hooks.get_axon_ntff_profile_hook()` when
    trace=True under axon; provide the module via sys.modules and wire the
    ctypes hook against the axon PJRT .so (same ABI trn_boot uses).
    """
    import sys
    import types
    import ctypes
    import contextlib

    if "antenv.axon_hooks" in sys.modules:
        return
    so_path = "/opt/axon/libaxon_pjrt.so"
    try:
        lib = ctypes.CDLL(so_path)
    except OSError:
        return
    if not hasattr(lib, "axon_start_nrt_profile"):
        return
    lib.axon_start_nrt_profile.argtypes = [
        ctypes.POINTER(ctypes.c_int64),
        ctypes.c_size_t,
    ]
    lib.axon_start_nrt_profile.restype = ctypes.c_int64
    lib.axon_stop_nrt_profile.argtypes = [ctypes.c_char_p]
    lib.axon_stop_nrt_profile.restype = ctypes.c_int64

    @contextlib.contextmanager
    def _hook(output_dir, device_ids):
        import jax

        jax.devices()
        if device_ids:
            ids = (ctypes.c_int64 * len(device_ids))(*device_ids)
            rc = lib.axon_start_nrt_profile(ids, len(device_ids))
        else:
            rc = lib.axon_start_nrt_profile(None, 0)
        if rc != 0:
            raise RuntimeError(f"axon_start_nrt_profile rc={rc}")
        try:
            yield
        finally:
            n = lib.axon_stop_nrt_profile(str(output_dir).encode())
            print(f"ntff profile: {n} file(s) written to {output_dir}")

    mod = types.ModuleType("antenv.axon_hooks")
    mod.get_axon_ntff_profile_hook = lambda: _hook
    mod.set_axon_ntff_profile_hook = lambda h: None
    sys.modules["antenv.axon_hooks"] = mod


_nc_cache = None


def _get_nc():
    global _nc_cache
    if _nc_cache is None:
        _nc_cache = _build()
    return _nc_cache


def _prep_inputs(x, w_qkv, w_o):
    x = np.asarray(x, dtype=np.float32)
    w_qkv = np.asarray(w_qkv, dtype=np.float32)
    w_o = np.asarray(w_o, dtype=np.float32)

    bf = ml_dtypes.bfloat16
    xT = x.reshape(R, D).T.astype(bf)                         # [D, R]
    # pre-tiled: chunk u block = [128 p, 8 ct x 512 j], p = d % 128
    xT2 = np.ascontiguousarray(
        xT.reshape(NC_T, 128, NT, 512).transpose(1, 2, 0, 3).reshape(128, R * NC_T)
    )
    woT_full = w_o.T  # [d, d'] contraction rows
    # head-half reorder: rows with (d mod 128) < 64 (h0 of each core), then >= 64
    dd = np.arange(D)
    order = np.concatenate([dd[(dd % 128) < 64], dd[(dd % 128) >= 64]])
    woT = woT_full[order].astype(bf)                          # [D, D]
    woT2 = np.ascontiguousarray(
        woT.reshape(NC_T, 128, D).transpose(1, 0, 2).reshape(128, NC_T * D)
    )

    w_q = w_qkv[0:D]
    w_k = w_qkv[D : 2 * D]
    w_v = w_qkv[2 * D : 3 * D]

    masks = np.zeros((4, 128, 512), ml_dtypes.bfloat16)
    kk = np.arange(128)[:, None]
    qq = np.arange(512)[None, :]
    for m in range(4):
        masks[m] = (qq >= kk + 128 * m).astype(ml_dtypes.bfloat16)
    masks = np.ascontiguousarray(masks.transpose(1, 0, 2).reshape(128, 2048))

    ident = np.eye(128, dtype=ml_dtypes.bfloat16)

    in_maps = []
    for c in range(N_CORES):
        h0, h1 = 2 * c, 2 * c + 1
        cols = []
        for w in (w_q, w_k, w_v):
            cols.append(w[DH * h0 : DH * h0 + DH])
            cols.append(w[DH * h1 : DH * h1 + DH])
        # [6*DH, D] rows: q_h0,q_h1,k_h0,k_h1,v_h0,v_h1 -> transpose to [D, 6*DH]
        w_slice = np.concatenate(cols, axis=0)
        wT = w_slice.T.astype(ml_dtypes.bfloat16)             # [D, 384]
        wT2 = np.ascontiguousarray(
            wT.reshape(NC_T, 128, 6 * DH).transpose(1, 0, 2).reshape(128, NC_T * 6 * DH)
        )
        in_maps.append(
            {
                "xT": xT2,
                "wT": wT2,
                "woT": woT2,
                "masks": masks,
                "ident": ident,
                "ones": np.ones((1, 64), np.float32),
            }
        )
    return in_maps


def kernel(x, w_qkv, w_o, _trace=False):
    if _trace:
        _install_ntff_shim()
    nc = _get_nc()
    in_maps = _prep_inputs(x, w_qkv, w_o)
    res = run_bass_kernel_spmd(
        nc, in_maps, list(range(N_CORES)), trace=_trace
    )
    out = np.concatenate(
        [res.results[c]["out"] for c in range(N_CORES)], axis=0
    )  # [R, D]
    out = out.reshape(B, S, D)
    if _trace:
        kernel.last_exec_time_ns = res.exec_time_ns
        kernel.last_results = res
    return out

